# revision 1
# baseline (speedup 1.0000x reference)
"""Mixture-of-Experts kernel for Trainium2 (8 NeuronCores).

Strategy (expert-parallel, sparse dispatch — per sharding hint):
  - Host computes the tiny gate (x @ Wg + bg, [16384, 8]), takes top-2,
    softmaxes the two logits, and dispatches tokens by expert id
    (the "all-to-all dispatch tokens by top-k expert id" sharding).
  - Core e receives: its expert's W1/W2/b1 (bf16/f32), the tokens routed
    to it (transposed, bf16, padded to capacity C), and per-token gate
    weights. It computes g * gelu(x @ W1 + b1) @ W2 on device.
  - Host scatter-adds the per-expert outputs back into token rows and
    adds the (gate-weighted) b2 term exactly: out += G @ b2.

Device kernel (per core), all matmuls bf16 with fp32 PSUM accumulation:
  mm1: hT[ht] = W1[:, ht].T @ xT          (H on PSUM partitions, tokens free)
  act: h[ht]  = gelu(hT[ht] + b1[ht])     (exact erf GELU, bias per partition)
  mm2: y[cs]  = h.T @ W2                  (tokens on PSUM partitions, D free)
  dve: y     *= g                         (per-partition = per-token scalar)
Weights stay resident in SBUF (16.8 MB bf16); tokens stream in 512-token
blocks.
"""

import numpy as np
import ml_dtypes

B, M, D, E, TOPK = 4096, 4, 1024, 8, 2
H = 4 * D
N = B * M
P = 128
CT = 512              # tokens per block
KD = D // P           # 8 k-tiles over D
HT = H // P           # 32 h-tiles over H

_BUILD_CACHE = {}


def _build(C, repeat=1):
    """Build + compile the per-core bass program for token capacity C.

    repeat>1 wraps the whole token-block loop in a hardware For_i that
    re-executes the body `repeat` times — used only by the timing harness
    (outputs are identical each iteration).
    """
    if (C, repeat) in _BUILD_CACHE:
        return _BUILD_CACHE[(C, repeat)]

    import concourse.bass as bass
    import concourse.mybir as mybir
    import concourse.tile as tile
    from concourse import bacc

    BF = mybir.dt.bfloat16
    F32 = mybir.dt.float32
    GELU = mybir.ActivationFunctionType.Gelu

    nc = bacc.Bacc(trn_type="TRN2", target_bir_lowering=False, debug=False)

    xT = nc.dram_tensor("xT", [KD, P, C], BF, kind="ExternalInput")
    w1 = nc.dram_tensor("w1", [KD, P, H], BF, kind="ExternalInput")
    w2 = nc.dram_tensor("w2", [HT, P, D], BF, kind="ExternalInput")
    b1t = nc.dram_tensor("b1t", [P, HT], F32, kind="ExternalInput")
    gt = nc.dram_tensor("gt", [P, C // P], F32, kind="ExternalInput")
    y = nc.dram_tensor("y", [C, D], F32, kind="ExternalOutput")

    y_r = y.rearrange("(ncs p) d -> ncs p d", p=P)

    # token blocks: full 512-wide blocks plus an optional 128-granular tail
    blocks = []
    off = 0
    while off < C:
        w = min(CT, C - off)
        blocks.append((off, w))
        off += w
    NDT = D // 512        # 2 D-tiles for mm2 free dim

    with tile.TileContext(nc) as tc:
        with (
            tc.tile_pool(name="weights", bufs=1) as wp,
            tc.tile_pool(name="xin", bufs=2) as xp,
            tc.tile_pool(name="hbuf", bufs=1) as hp,
            tc.tile_pool(name="yout", bufs=2) as yp,
            tc.tile_pool(name="ps_h", bufs=4, space="PSUM") as ph,
            tc.tile_pool(name="ps_o", bufs=2, space="PSUM") as po,
        ):
            # prologue loads, ordered so the first matmul can start earliest:
            # block-0 x + W1 first, then bias/gates, then W2 (only needed
            # ~55us in, hidden under block-0 mm1).
            xblk0 = []
            for k in range(KD):
                t = xp.tile([P, blocks[0][1]], BF, tag=f"x{k}")
                nc.scalar.dma_start(t, xT[k][:, 0:blocks[0][1]])
                xblk0.append(t)
            # W1 loaded in H-quarters, k-interleaved: after one quarter
            # (~2.1 MB) the first 8 ht-tiles have all 8 k-slices, so block-0
            # mm1 can stream while the rest of W1 arrives.
            w1sb = [
                wp.tile([P, H], BF, tag=f"w1_{k}", name=f"w1_{k}")
                for k in range(KD)
            ]
            HQ = H // 2
            for q in range(2):
                for k in range(KD):
                    nc.sync.dma_start(
                        w1sb[k][:, q * HQ:(q + 1) * HQ],
                        w1[k][:, q * HQ:(q + 1) * HQ],
                    )
            b1sb = wp.tile([P, HT], F32, tag="b1t")
            nc.sync.dma_start(b1sb, b1t.ap())
            gtsb = wp.tile([P, C // P], F32, tag="gt")
            nc.sync.dma_start(gtsb, gt.ap())
            w2sb = []
            for ht in range(HT):
                t = wp.tile([P, D], BF, tag=f"w2_{ht}")
                nc.sync.dma_start(t, w2[ht])
                w2sb.append(t)

            import contextlib
            loop_ctx = (
                tc.For_i(0, repeat, 1) if repeat > 1 else contextlib.nullcontext()
            )
            with loop_ctx:
              for blk, (c0, cw) in enumerate(blocks):
                ncs = cw // P
                if blk == 0 and repeat == 1:
                    xblk = xblk0
                else:
                    xblk = []
                    for k in range(KD):
                        t = xp.tile([P, cw], BF, tag=f"x{k}")
                        nc.scalar.dma_start(t, xT[k][:, c0:c0 + cw])
                        xblk.append(t)

                # mm1 + gelu: h_all[ht] = gelu(W1[:,ht].T @ x + b1[ht])
                h_all = hp.tile([P, HT, cw], BF, tag="h")
                for ht in range(HT):
                    psum_h = ph.tile([P, cw], F32, tag="ph")
                    for k in range(KD):
                        nc.tensor.matmul(
                            psum_h,
                            w1sb[k][:, ht * P:(ht + 1) * P],
                            xblk[k],
                            start=(k == 0),
                            stop=(k == KD - 1),
                        )
                    nc.scalar.activation(
                        h_all[:, ht], psum_h, GELU, bias=b1sb[:, ht:ht + 1]
                    )

                # mm2 + gate scale: y[cs] = g * (h.T @ W2).
                # dt innermost: both 512-wide D-tiles share the same
                # stationary h-slice, so the duplicate LDWEIGHTS is stripped
                # by _dedup_ldweights below.
                for cs in range(ncs):
                    pots = [po.tile([P, 512], F32, tag=f"po{dt}", name=f"po{dt}")
                            for dt in range(NDT)]
                    for ht in range(HT):
                        for dt in range(NDT):
                            nc.tensor.matmul(
                                pots[dt],
                                h_all[:, ht, cs * P:(cs + 1) * P],
                                w2sb[ht][:, dt * 512:(dt + 1) * 512],
                                start=(ht == 0),
                                stop=(ht == HT - 1),
                            )
                    for dt in range(NDT):
                        ysb = yp.tile([P, 512], F32, tag=f"y{dt}")
                        nc.vector.tensor_scalar_mul(
                            ysb, pots[dt], gtsb[:, c0 // P + cs:c0 // P + cs + 1]
                        )
                        nc.sync.dma_start(
                            y_r[c0 // P + cs][:, dt * 512:(dt + 1) * 512], ysb
                        )
    _dedup_ldweights(nc)
    nc.compile()
    _BUILD_CACHE[(C, repeat)] = nc
    return nc


def _ap_key(arg):
    """Stable identity key for an instruction AP argument, or None."""
    try:
        ap = arg.bass_ap if hasattr(arg, "bass_ap") else arg
        t = ap.tensor
        return (t.name, ap.offset, tuple(map(tuple, ap.ap)))
    except Exception:
        return None


def _dedup_ldweights(nc):
    """Drop an InstLdweights when the immediately-preceding PE instruction
    sequence already loaded the identical weights AP (PE weight state is
    sticky until the next LDWEIGHTS). Only sync-free duplicates are dropped.
    """
    import concourse.mybir as mybir

    n_del = 0
    for blk in nc.m.functions[0].blocks:
        insts = list(blk.instructions)
        keep = []
        last_key = None
        for inst in insts:
            tn = type(inst).__name__
            if tn == "InstLdweights":
                key = _ap_key(inst.ins[0])
                si = inst.sync_info
                clean = not (si and (si.on_wait or si.on_update))
                if key is not None and key == last_key and clean:
                    n_del += 1
                    continue
                last_key = key
            elif tn != "InstMatmult" and getattr(inst, "engine", None) == mybir.EngineType.PE:
                last_key = None
            keep.append(inst)
        if len(keep) != len(insts):
            while len(blk.instructions):
                blk.instructions.pop()
            for inst in keep:
                blk.instructions.append(inst)
    return n_del


def _route(xf, Wg, bg):
    """Top-2 gating on host. Returns (idx, gate) per expert and dense G."""
    logits = xf @ Wg + bg                      # [N, E] f32
    n = logits.shape[0]
    ar = np.arange(n)
    i1 = np.argmax(logits, axis=1)
    v1 = logits[ar, i1]
    masked = logits.copy()
    masked[ar, i1] = -np.inf
    i2 = np.argmax(masked, axis=1)
    v2 = masked[ar, i2]
    e2 = np.exp(v2 - v1)
    wt1 = 1.0 / (1.0 + e2)
    wt2 = e2 / (1.0 + e2)
    G = np.zeros_like(logits)
    G[ar, i1] = wt1
    G[ar, i2] = wt2
    idxs, gates = [], []
    for e in range(E):
        idx = np.nonzero((i1 == e) | (i2 == e))[0]
        idxs.append(idx)
        gates.append(G[idx, e].astype(np.float32))
    return idxs, gates, G.astype(np.float32)


def kernel(_trace=False, **inputs):
    x = np.asarray(inputs["x"], dtype=np.float32)
    Wg = np.asarray(inputs["Wg"], dtype=np.float32)
    bg = np.asarray(inputs["bg"], dtype=np.float32)
    W1 = np.asarray(inputs["W1"], dtype=np.float32)
    b1 = np.asarray(inputs["b1"], dtype=np.float32)
    W2 = np.asarray(inputs["W2"], dtype=np.float32)
    b2 = np.asarray(inputs["b2"], dtype=np.float32)

    Bn, Mn, Dn = x.shape
    n = Bn * Mn
    xf = x.reshape(n, Dn)

    idxs, gates, G = _route(xf, Wg, bg)

    C = max(len(i) for i in idxs)
    C = ((C + P - 1) // P) * P

    bf16 = ml_dtypes.bfloat16
    xf_bf = xf.astype(bf16)

    in_maps = []
    for e in range(E):
        ne = len(idxs[e])
        xTe = np.zeros((Dn, C), dtype=bf16)
        xTe[:, :ne] = xf_bf[idxs[e]].T
        ge = np.zeros((C,), dtype=np.float32)
        ge[:ne] = gates[e]
        in_maps.append({
            "xT": np.ascontiguousarray(xTe.reshape(KD, P, C)),
            "w1": np.ascontiguousarray(W1[e].astype(bf16).reshape(KD, P, H)),
            "w2": np.ascontiguousarray(W2[e].astype(bf16).reshape(HT, P, D)),
            "b1t": np.ascontiguousarray(b1[e].reshape(HT, P).T),
            "gt": np.ascontiguousarray(ge.reshape(C // P, P).T),
        })

    nc = _build(C)

    from concourse.bass_utils import run_bass_kernel_spmd
    res = run_bass_kernel_spmd(
        nc, in_maps, core_ids=list(range(E)), trace=_trace
    )

    out = G @ b2                               # gate-weighted b2, exact
    for e in range(E):
        ne = len(idxs[e])
        out[idxs[e]] += res.results[e]["y"][:ne]

    if _trace:
        return out.reshape(Bn, Mn, Dn), res
    return out.reshape(Bn, Mn, Dn)



# revision 3
# speedup vs baseline: 13.4723x; 13.4723x over previous
"""Mixture-of-Experts kernel for Trainium2 (8 NeuronCores).

Strategy (expert-parallel, sparse dispatch — per sharding hint):
  - Host computes the tiny gate (x @ Wg + bg, [16384, 8]), takes top-2,
    softmaxes the two logits, and dispatches tokens by expert id
    (the "all-to-all dispatch tokens by top-k expert id" sharding).
  - Core e receives: its expert's W1/W2/b1 (bf16/f32), the tokens routed
    to it (transposed, bf16, padded to capacity C), and per-token gate
    weights. It computes g * gelu(x @ W1 + b1) @ W2 on device.
  - Host scatter-adds the per-expert outputs back into token rows and
    adds the (gate-weighted) b2 term exactly: out += G @ b2.

Device kernel (per core), all matmuls bf16 with fp32 PSUM accumulation,
two phases with h spilled to a DRAM scratch (h for all C tokens does
not fit in SBUF alongside the weights):

Phase 1 (mm1+gelu), k-outer for LDWEIGHTS amortization:
  for ht: for half: for k: LDW(w1[k,ht]); 4x MM into psum[chunk]
  -> one LDWEIGHTS per 4-5 N=512 matmuls (vs 1:1 column-wise).
  gelu(+b1) drains each bank into h[ht] [128, C] which DMAs to DRAM.
  The 128-token tail rides in half B's k-loop on a 9th psum rotation.

Phase 2 (mm2+gate), chunk-outer (512 tokens), ht-inner:
  h chunk tiles stream back from DRAM (chunk 0 prefetches during
  phase 1); stationary h[ht, cs] is shared by the two 512-wide D tiles
  of W2 (duplicate LDWEIGHTS removed by _dedup_ldweights), accumulating
  32 ht steps into 8 PSUM banks (4 cs x 2 dt). DVE applies the
  per-token gate and y DMAs out.

W2 is not separately resident: its [128,1024] column slabs DMA into the
same SBUF tiles that held W1 slabs, which die ht-group by ht-group
during phase 1 (tile tag rotation inserts the WAR waits). The prologue
round-robins the PE-critical (w1 slab, x half) pairs over all three
DMA-capable queues (ACT/SP/gpsimd); it is DMA-bandwidth-floor bound.
"""

import numpy as np
import ml_dtypes

B, M, D, E, TOPK = 4096, 4, 1024, 8, 2
H = 4 * D
N = B * M
P = 128
CT = 512              # tokens per chunk (= one fp32 PSUM bank)
KD = D // P           # 8 k-tiles over D
HT = H // P           # 32 h-tiles over H

_BUILD_CACHE = {}


def _build(C, repeat=1):
    """Build + compile the per-core bass program for token capacity C.

    repeat>1 python-unrolls the whole program body `repeat` times (used
    only by timing harnesses to measure steady-state per-iteration cost).
    """
    if (C, repeat) in _BUILD_CACHE:
        return _BUILD_CACHE[(C, repeat)]

    import concourse.mybir as mybir
    import concourse.tile as tile
    from concourse import bacc

    BF = mybir.dt.bfloat16
    F32 = mybir.dt.float32
    GELU = mybir.ActivationFunctionType.Gelu

    NCH = C // CT        # full 512-token chunks (8 for C=4224)
    TAIL = C - NCH * CT  # 128 for C=4224
    # phase 1 runs two 4-chunk halves over the 8 psum banks; the tail
    # (up to 384 tokens, < 1 bank of fp32) rides a 9th psum rotation
    assert NCH == 8 and TAIL % P == 0 and TAIL <= 384

    nc = bacc.Bacc(trn_type="TRN2", target_bir_lowering=False, debug=False)

    xT = nc.dram_tensor("xT", [KD, P, C], BF, kind="ExternalInput")
    w1 = nc.dram_tensor("w1", [KD, P, H], BF, kind="ExternalInput")
    w2 = nc.dram_tensor("w2", [HT, P, D], BF, kind="ExternalInput")
    b1t = nc.dram_tensor("b1t", [P, HT], F32, kind="ExternalInput")
    gt = nc.dram_tensor("gt", [P, C // P], F32, kind="ExternalInput")
    y = nc.dram_tensor("y", [C, D], F32, kind="ExternalOutput")
    y_r = y.rearrange("(ncs p) d -> ncs p d", p=P)

    with tile.TileContext(nc) as tc:
        with (
            tc.tile_pool(name="wslab", bufs=1) as wp,
            tc.tile_pool(name="xin", bufs=1) as xp,
            tc.tile_pool(name="hout", bufs=2) as hop,
            tc.tile_pool(name="hin", bufs=1) as hip,
            tc.tile_pool(name="yout", bufs=1) as yp,
            tc.tile_pool(name="small", bufs=1) as sp,
            tc.tile_pool(name="hdram", bufs=1, space="DRAM") as dp,
            tc.tile_pool(name="ps", bufs=1, space="PSUM") as pp,
        ):
            for _rep in range(repeat):
                # ---- input loads ---------------------------------------
                # ht=0's k-step needs slab(k, g=0) AND x[k] halfA in
                # lockstep (~0.85us/k of PE work). Round-robin the critical
                # (slab, xA) pairs over all three DMA-capable queues.
                HA = 4 * CT
                queues = [nc.scalar, nc.sync, nc.gpsimd]
                b1sb = sp.tile([P, HT], F32, tag="b1t", name="b1sb")
                gtsb = sp.tile([P, C // P], F32, tag="gt", name="gtsb")
                xk = [None] * KD
                slab = [[None] * (HT // 8) for _ in range(KD)]
                for k in range(KD):
                    q = queues[k % 3]
                    t = wp.tile([P, 1024], BF, tag=f"sl{k}_0", name=f"w1s{k}_0")
                    q.dma_start(t, w1[k][:, 0:1024])
                    slab[k][0] = t
                    tx = xp.tile([P, C], BF, tag=f"x{k}", name=f"x{k}")
                    q.dma_start(tx[:, 0:HA], xT[k][:, 0:HA])
                    xk[k] = tx
                    if k == 1:
                        nc.sync.dma_start(b1sb, b1t.ap())
                        nc.sync.dma_start(gtsb, gt.ap())
                for k in range(KD):
                    queues[k % 3].dma_start(xk[k][:, HA:C], xT[k][:, HA:C])
                if _rep == 0:
                    # Preload the Gelu ACT table while the prologue DMAs run.
                    warm = sp.tile([P, 1], F32, tag="warm", name="warm")
                    nc.scalar.activation(warm, b1sb[:, 0:1], GELU)
                for g in range(1, HT // 8):
                    for k in range(KD):
                        t = wp.tile([P, 1024], BF, tag=f"sl{k}_{g}",
                                    name=f"w1s{k}_{g}")
                        nc.sync.dma_start(t, w1[k][:, 1024 * g:1024 * (g + 1)])
                        slab[k][g] = t

                # ---- phase 1: h[ht] = gelu(W1[:,ht].T @ x + b1), spill --
                # DRAM h scratch: 8 tiles of 4 ht-rows so phase-2 reads only
                # wait on the 4 writes they cover (chunk-0 prefetch overlaps
                # the phase-1 tail).
                hd = [
                    dp.tile([P, 4, C], BF, tag=f"hd{g2}", name=f"hd{g2}")
                    for g2 in range(HT // 4)
                ]
                # w2 slabs land in dead w1 slab tiles: w2[ht] -> slab tag
                # (k=ht%8, g=ht//8), free after phase-1 ht = 8*(ht//8)+7.
                w2sb = [None] * HT

                for ht in range(HT):
                    g, col = ht // 8, (ht % 8) * P
                    hout = hop.tile([P, C], BF, tag="hout", name="hout")
                    for half in range(2):
                        chunks = range(4 * half, 4 * half + 4)
                        pst = [
                            pp.tile([P, CT], F32, tag=f"ps{c % 4 + 4 * half}",
                                    name=f"ps{c}")
                            for c in chunks
                        ]
                        tail_ps = None
                        if half == 1 and TAIL:
                            tail_ps = pp.tile([P, TAIL], F32, tag="ps0",
                                              name="pstail")
                        for k in range(KD):
                            stat = slab[k][g][:, col:col + P]
                            for i, c in enumerate(chunks):
                                nc.tensor.matmul(
                                    pst[i], stat,
                                    xk[k][:, CT * c:CT * (c + 1)],
                                    start=(k == 0), stop=(k == KD - 1),
                                )
                            if tail_ps is not None:
                                nc.tensor.matmul(
                                    tail_ps, stat,
                                    xk[k][:, NCH * CT:C],
                                    start=(k == 0), stop=(k == KD - 1),
                                )
                        for i, c in enumerate(chunks):
                            nc.scalar.activation(
                                hout[:, CT * c:CT * (c + 1)], pst[i], GELU,
                                bias=b1sb[:, ht:ht + 1],
                            )
                        if tail_ps is not None:
                            nc.scalar.activation(
                                hout[:, NCH * CT:C], tail_ps, GELU,
                                bias=b1sb[:, ht:ht + 1],
                            )
                    nc.gpsimd.dma_start(hd[ht // 4][:, ht % 4], hout)
                    if ht % 8 == 7:
                        for htp in range(8 * (ht // 8), 8 * (ht // 8) + 8):
                            t = wp.tile([P, 1024], BF,
                                        tag=f"sl{htp % 8}_{htp // 8}",
                                        name=f"w2s{htp}")
                            nc.sync.dma_start(t, w2[htp])
                            w2sb[htp] = t

                # ---- phase 2: y[chunk] = gate * (h.T @ W2) --------------
                for c in range(NCH + (1 if TAIL else 0)):
                    w = CT if c < NCH else TAIL
                    ncs = w // P
                    hp = []
                    for g2 in range(HT // 4):
                        t = hip.tile([P, 4, w], BF, tag=f"hp{g2}",
                                     name=f"hp{g2}")
                        nc.gpsimd.dma_start(t, hd[g2][:, :, CT * c:CT * c + w])
                        hp.append(t)
                    po = [
                        [
                            pp.tile([P, CT], F32, tag=f"ps{cs * 2 + dt}",
                                    name=f"po{cs}_{dt}")
                            for dt in range(2)
                        ]
                        for cs in range(ncs)
                    ]
                    for ht in range(HT):
                        hpt = hp[ht // 4][:, ht % 4]
                        for cs in range(ncs):
                            stat = hpt[:, cs * P:(cs + 1) * P]
                            for dt in range(2):
                                nc.tensor.matmul(
                                    po[cs][dt], stat,
                                    w2sb[ht][:, 512 * dt:512 * (dt + 1)],
                                    start=(ht == 0), stop=(ht == HT - 1),
                                )
                    for cs in range(ncs):
                        for dt in range(2):
                            ysb = yp.tile([P, CT], F32, tag=f"y{cs * 2 + dt}",
                                          name=f"ysb{cs}_{dt}")
                            nc.vector.tensor_scalar_mul(
                                ysb, po[cs][dt],
                                gtsb[:, 4 * c + cs:4 * c + cs + 1],
                            )
                            nc.sync.dma_start(
                                y_r[4 * c + cs][:, 512 * dt:512 * (dt + 1)],
                                ysb,
                            )
    _dedup_ldweights(nc)
    nc.compile()
    _BUILD_CACHE[(C, repeat)] = nc
    return nc


def _ap_key(arg):
    """Stable identity key for an instruction AP argument, or None."""
    try:
        ap = arg.bass_ap if hasattr(arg, "bass_ap") else arg
        t = ap.tensor
        return (t.name, ap.offset, tuple(map(tuple, ap.ap)))
    except Exception:
        return None


def _dedup_ldweights(nc):
    """Drop an InstLdweights when the immediately-preceding PE instruction
    sequence already loaded the identical weights AP (PE weight state is
    sticky until the next LDWEIGHTS). Only sync-free duplicates are dropped.
    """
    import concourse.mybir as mybir

    n_del = 0
    for blk in nc.m.functions[0].blocks:
        insts = list(blk.instructions)
        keep = []
        last_key = None
        for inst in insts:
            tn = type(inst).__name__
            if tn == "InstLdweights":
                key = _ap_key(inst.ins[0])
                si = inst.sync_info
                clean = not (si and (si.on_wait or si.on_update))
                if key is not None and key == last_key and clean:
                    n_del += 1
                    continue
                last_key = key
            elif tn != "InstMatmult" and getattr(inst, "engine", None) == mybir.EngineType.PE:
                last_key = None
            keep.append(inst)
        if len(keep) != len(insts):
            while len(blk.instructions):
                blk.instructions.pop()
            for inst in keep:
                blk.instructions.append(inst)
    return n_del


def _route(xf, Wg, bg):
    """Top-2 gating on host. Returns (idx, gate) per expert and dense G."""
    logits = xf @ Wg + bg                      # [N, E] f32
    n = logits.shape[0]
    ar = np.arange(n)
    i1 = np.argmax(logits, axis=1)
    v1 = logits[ar, i1]
    masked = logits.copy()
    masked[ar, i1] = -np.inf
    i2 = np.argmax(masked, axis=1)
    v2 = masked[ar, i2]
    e2 = np.exp(v2 - v1)
    wt1 = 1.0 / (1.0 + e2)
    wt2 = e2 / (1.0 + e2)
    G = np.zeros_like(logits)
    G[ar, i1] = wt1
    G[ar, i2] = wt2
    idxs, gates = [], []
    for e in range(E):
        idx = np.nonzero((i1 == e) | (i2 == e))[0]
        idxs.append(idx)
        gates.append(G[idx, e].astype(np.float32))
    return idxs, gates, G.astype(np.float32)


def kernel(_trace=False, **inputs):
    x = np.asarray(inputs["x"], dtype=np.float32)
    Wg = np.asarray(inputs["Wg"], dtype=np.float32)
    bg = np.asarray(inputs["bg"], dtype=np.float32)
    W1 = np.asarray(inputs["W1"], dtype=np.float32)
    b1 = np.asarray(inputs["b1"], dtype=np.float32)
    W2 = np.asarray(inputs["W2"], dtype=np.float32)
    b2 = np.asarray(inputs["b2"], dtype=np.float32)

    Bn, Mn, Dn = x.shape
    n = Bn * Mn
    xf = x.reshape(n, Dn)

    idxs, gates, G = _route(xf, Wg, bg)

    C = max(len(i) for i in idxs)
    C = ((C + P - 1) // P) * P
    # the device program is specialized for 8 full chunks + 128 tail
    C = max(C, 8 * CT + P)

    bf16 = ml_dtypes.bfloat16
    xf_bf = xf.astype(bf16)

    in_maps = []
    for e in range(E):
        ne = len(idxs[e])
        xTe = np.zeros((Dn, C), dtype=bf16)
        xTe[:, :ne] = xf_bf[idxs[e]].T
        ge = np.zeros((C,), dtype=np.float32)
        ge[:ne] = gates[e]
        in_maps.append({
            "xT": np.ascontiguousarray(xTe.reshape(KD, P, C)),
            "w1": np.ascontiguousarray(W1[e].astype(bf16).reshape(KD, P, H)),
            "w2": np.ascontiguousarray(W2[e].astype(bf16).reshape(HT, P, D)),
            "b1t": np.ascontiguousarray(b1[e].reshape(HT, P).T),
            "gt": np.ascontiguousarray(ge.reshape(C // P, P).T),
        })

    nc = _build(C)

    from concourse.bass_utils import run_bass_kernel_spmd
    res = run_bass_kernel_spmd(
        nc, in_maps, core_ids=list(range(E)), trace=_trace
    )

    out = G @ b2                               # gate-weighted b2, exact
    for e in range(E):
        ne = len(idxs[e])
        out[idxs[e]] += res.results[e]["y"][:ne]

    if _trace:
        return out.reshape(Bn, Mn, Dn), res
    return out.reshape(Bn, Mn, Dn)
